# revision 7
# baseline (speedup 1.0000x reference)
"""Trainium2 Bass kernel: multi-head self-attention over images (1x1-conv QKV).

Problem: x [4, 256, 64, 64], w_qkv [384, 256], w_out [256, 128], b_out [256].
  qkv = w_qkv @ x_flat ; per-head (h=4, d=32) softmax attention over n=4096 ;
  out = w_out @ heads + b_out.

Sharding across 8 cores: (batch, query-half) pairs -> each core handles one
batch's K/V over the full 4096 positions and attention + output projection
for 2048 of its query positions.  Outputs are disjoint slices; the host
only concatenates.

Per-core dataflow (v3 = head-pair passes, N=512 matmuls):
  - All matmul operands are bf16 (fp32 costs 4 cycles/row on the PE, bf16
    one); accumulation stays fp32 in PSUM.  Host pre-converts inputs.
  - Work is split into 4 i-chunks of 512 columns, each processed in two
    head-pair passes (plane 0 = heads 0,1; plane 1 = heads 2,3).  Per
    (pass, j-tile): 2 sim matmuls [K=32 x N=512] at array rows 0-31/64-95
    writing the two banks of a [128, 2, 512] PSUM tile, one exp on ScalarE
    (N=1024, PSUM -> SBUF bf16), 2 AV matmuls accumulating into a single
    av bank at partition rows 0-32 / 64-96 (ones column at d=32 gives the
    softmax denominator).  N=512 halves the PE instruction count vs an
    ic=256 layout - HW probes showed per-instruction overheads on PE and
    ACT are first-order costs.
  - Softmax division: row-sums broadcast across partitions with K=1
    matmuls against a constant mask, reciprocal + multiply on VectorE.
  - Per chunk the output projection accumulates both head-pair passes
    (host-prescrambled w_out halves) + bias.
  - Prologue K/Q/V projections and input DMAs are split and interleaved
    into the first pass so ScalarE starts ~3us in; each pass's division
    is deferred into the next pass's first j-tiles so ScalarE never idles.
"""

import sys

import numpy as np

for _p in ("/opt/trn_rl_repo",):
    if _p not in sys.path:
        sys.path.insert(0, _p)

import ml_dtypes

BF16 = ml_dtypes.bfloat16

HEADS = 4
DH = 32
DIM = 256
HID = HEADS * DH  # 128
B = 4
N_CORES = 8
NJ = 4096  # full context per batch (64*64)
NI = NJ // 2  # queries per core
IC = 512  # i-chunk (columns per inner tile)


def build_attn(tc, out_ap, in_aps, nj, ni, ic, repeat=1):
    """Emit the per-core attention program.

    out_ap: DRAM AP [256, ni] fp32
    in_aps: dict with DRAM APs:
        x_kv  [256, nj]   bf16 batch image, channels-major
        x_q   [256, ni]   bf16 this core's query columns
        w_qkvt [256, 640] bf16 (w_qkv with q-scale folded in).T, prescrambled
        w_out_a/w_out_b [128, 256] bf16 prescrambled output weights
        b2    [128, 2]    fp32 bias, b2[p, mc] = b_out[mc*128 + p]
    """
    import concourse.tile as tile  # noqa: F401
    from concourse import mybir

    nc = tc.nc
    f32 = mybir.dt.float32
    bf16 = mybir.dt.bfloat16
    Exp = mybir.ActivationFunctionType.Exp
    jt_n = nj // 128
    nic = ni // ic
    assert nj % 512 == 0 and ni % 512 == 0 and ni % ic == 0

    from contextlib import ExitStack

    with ExitStack() as ctx:
        const = ctx.enter_context(tc.tile_pool(name="const", bufs=1))
        sim_pool = ctx.enter_context(tc.tile_pool(name="simps", bufs=2, space="PSUM"))
        av_pool = ctx.enter_context(tc.tile_pool(name="avps", bufs=2, space="PSUM"))
        work_pool = ctx.enter_context(tc.tile_pool(name="workps", bufs=2, space="PSUM"))
        exp_pool = ctx.enter_context(tc.tile_pool(name="expsb", bufs=7))
        sb_pool = ctx.enter_context(tc.tile_pool(name="sb", bufs=2))

        # ---------------- persistent SBUF ----------------
        x_sb = const.tile([128, 2, nj], bf16, tag="x_sb")
        xq_sb = const.tile([128, 2, ni], bf16, tag="xq_sb")
        wqkvt_sb = const.tile([128, 2, 5 * HID], bf16, tag="wqkvt")
        wouta_sb = const.tile([128, 256], bf16, tag="wouta")
        woutb_sb = const.tile([128, 256], bf16, tag="woutb")
        b_sb = const.tile([128, 2], f32, tag="b_sb")
        mask_sb = const.tile([128, 128], bf16, tag="mask")
        k_sb = const.tile([128, 2, nj], bf16, tag="k_sb")
        q_sb = const.tile([128, 2, ni], bf16, tag="q_sb")
        vt_sb = const.tile([128, jt_n, HEADS, DH + 1], bf16, tag="vt_sb")
        # normalized head outputs; [hd, buf, pass, ic]; rows 33-63/97-127 stay 0
        oh_sb = const.tile([128, 2, 2, ic], bf16, tag="oh_sb")

        # ---------------- input DMAs (split + ordered so compute starts
        # early: qkv weights and the first x/xq chunks first) -------------
        xr = in_aps["x_kv"].rearrange("(c p) n -> p c n", p=128)
        xqr = in_aps["x_q"].rearrange("(c p) n -> p c n", p=128)

        def xdma(jc):
            s = slice(jc * 512, (jc + 1) * 512)
            nc.sync.dma_start(x_sb[:, :, s], xr[:, :, s])

        def xqdma(qc):
            s = slice(qc * 512, (qc + 1) * 512)
            nc.sync.dma_start(xq_sb[:, :, s], xqr[:, :, s])

        nc.sync.dma_start(
            wqkvt_sb[:], in_aps["w_qkvt"].rearrange("(c p) m -> p c m", p=128)
        )
        xdma(0)
        xqdma(0)
        nc.sync.dma_start(wouta_sb[:], in_aps["w_out_a"][:])
        nc.sync.dma_start(woutb_sb[:], in_aps["w_out_b"][:])
        nc.sync.dma_start(b_sb[:], in_aps["b2"][:])
        for jc in range(1, nj // 512):
            xdma(jc)
        for qc in range(1, ni // 512):
            xqdma(qc)

        out_r = out_ap.rearrange("(m p) n -> p m n", p=128)

        # ---------------- emission helpers ----------------
        def kproj(pl, jc):
            ps = work_pool.tile([128, 2, 256], f32, tag="work")
            psf = ps[:].rearrange("p a b -> p (a b)")
            for cs in range(2):
                nc.tensor.matmul(
                    psf,
                    lhsT=wqkvt_sb[:, cs, 2 * HID + 128 * pl : 2 * HID + 128 * (pl + 1)],
                    rhs=x_sb[:, cs, jc * 512 : (jc + 1) * 512],
                    start=(cs == 0),
                    stop=(cs == 1),
                )
            nc.vector.tensor_copy(k_sb[:, pl, jc * 512 : (jc + 1) * 512], psf)

        def qproj(pl, qc):
            ps = work_pool.tile([128, 2, 256], f32, tag="work")
            psf = ps[:].rearrange("p a b -> p (a b)")
            for cs in range(2):
                nc.tensor.matmul(
                    psf,
                    lhsT=wqkvt_sb[:, cs, 128 * pl : 128 * (pl + 1)],
                    rhs=xq_sb[:, cs, qc * 512 : (qc + 1) * 512],
                    start=(cs == 0),
                    stop=(cs == 1),
                )
            nc.vector.tensor_copy(q_sb[:, pl, qc * 512 : (qc + 1) * 512], psf)

        def vproj(jt):
            # V^T: vt_sb[j, h, d] (+ ones column at d=DH)
            ps = work_pool.tile([128, 2, 256], f32, tag="work")
            psf = ps[:].rearrange("p a b -> p (a b)")
            for cs in range(2):
                nc.tensor.matmul(
                    psf[:, 0:HID],
                    lhsT=x_sb[:, cs, jt * 128 : (jt + 1) * 128],
                    rhs=wqkvt_sb[:, cs, 4 * HID : 5 * HID],
                    start=(cs == 0),
                    stop=(cs == 1),
                )
            nc.vector.tensor_copy(
                vt_sb[:, jt, :, 0:DH],
                psf[:, 0:HID].rearrange("p (h d) -> p h d", h=HEADS),
            )

        def sim_exp(st, jt):
            """sim matmuls + exp for one j-tile of one pass; returns exp tile."""
            av_ps, ici, hp = st
            i_sl = slice(ici * ic, (ici + 1) * ic)
            sim_ps = sim_pool.tile([128, 2, ic], f32, tag="sim")
            # the two heads of the pass hit different PSUM banks at array
            # row positions 0 and 64, so they run concurrently.
            for s in range(2):
                nc.tensor.matmul(
                    sim_ps[:, s, :],
                    lhsT=k_sb[64 * s : 64 * s + 32, hp, jt * 128 : (jt + 1) * 128],
                    rhs=q_sb[64 * s : 64 * s + 32, hp, i_sl],
                    start=True,
                    stop=True,
                    tile_position=(64 * s, 0),
                )
            ex = exp_pool.tile([128, 2, ic], bf16, tag="exp")
            nc.scalar.activation(ex[:], sim_ps[:], Exp)
            return ex

        def av_mms(st, jt, ex):
            av_ps, ici, hp = st
            for s in range(2):
                pos = 64 * s
                # the two accumulation groups live at disjoint partition
                # ranges of one bank; the sim's group check is partition-
                # blind, so it must be skipped (HW-legal).
                nc.tensor.matmul(
                    av_ps[pos : pos + DH + 1, 0:ic],
                    lhsT=vt_sb[:, jt, 2 * hp + s, :],
                    rhs=ex[:, s, :],
                    start=(jt == 0),
                    stop=(jt == jt_n - 1),
                    skip_group_check=True,
                )

        def epilogue_div(st):
            """Softmax division: av -> normalized oh_sb (frees the av bank)."""
            av_ps, ici, hp = st
            buf = ici % 2
            sums = sb_pool.tile([128, ic], bf16, tag="sums")
            nc.vector.tensor_copy(sums[32:33], av_ps[32:33, 0:ic])
            nc.vector.tensor_copy(sums[96:97], av_ps[96:97, 0:ic])
            bc_e = work_pool.tile([128, 2, 256], f32, tag="work")
            bc_o = work_pool.tile([128, 2, 256], f32, tag="work")
            bce = bc_e[:].rearrange("p a b -> p (a b)")
            bco = bc_o[:].rearrange("p a b -> p (a b)")
            nc.tensor.matmul(
                bce,
                lhsT=mask_sb[32:33, :],
                rhs=sums[32:33, :],
                start=True,
                stop=True,
                tile_position=(32, 0),
            )
            nc.tensor.matmul(
                bco,
                lhsT=mask_sb[96:97, :],
                rhs=sums[96:97, :],
                start=True,
                stop=True,
                tile_position=(96, 0),
            )
            recip = sb_pool.tile([128, ic], f32, tag="recip")
            nc.vector.reciprocal(recip[0:33], bce[0:33])
            nc.vector.reciprocal(recip[64:97], bco[64:97])
            nc.vector.tensor_mul(oh_sb[0:33, buf, hp], av_ps[0:33, 0:ic], recip[0:33])
            nc.vector.tensor_mul(oh_sb[64:97, buf, hp], av_ps[64:97, 0:ic], recip[64:97])

        def epilogue_out(ici):
            """Output projection + bias + store for chunk ici (both passes)."""
            buf = ici % 2
            for mc in range(2):
                pout = work_pool.tile([128, 2, 256], f32, tag="work")
                pf = pout[:].rearrange("p a b -> p (a b)")
                nc.tensor.matmul(
                    pf,
                    lhsT=wouta_sb[:, mc * 128 : (mc + 1) * 128],
                    rhs=oh_sb[:, buf, 0, :],
                    start=True,
                    stop=False,
                )
                nc.tensor.matmul(
                    pf,
                    lhsT=woutb_sb[:, mc * 128 : (mc + 1) * 128],
                    rhs=oh_sb[:, buf, 1, :],
                    start=False,
                    stop=True,
                )
                final = sb_pool.tile([128, ic], f32, tag="final")
                nc.vector.tensor_scalar_add(final[:], pf, b_sb[:, mc : mc + 1])
                nc.sync.dma_start(
                    out_r[:, mc, ici * ic : (ici + 1) * ic], final[:]
                )

        DEFER = 4  # j-tiles of AV deferred past the previous pass's division

        for _rep in range(repeat):
            # ---------------- constants ----------------
            nc.vector.memset(mask_sb[:], 0.0)
            nc.vector.memset(mask_sb[32:33, 0:33], 1.0)
            nc.vector.memset(mask_sb[96:97, 64:97], 1.0)
            nc.vector.memset(vt_sb[:, :, :, DH : DH + 1], 1.0)
            # rows 33-63 / 97-127 stay zero; live rows are rewritten every chunk
            nc.vector.memset(oh_sb[:], 0.0)

            # minimal prologue: K and Q for the first j/i chunks only; the
            # rest is interleaved into the first pass so ScalarE starts early.
            kproj(0, 0)
            qproj(0, 0)

            # ---------------- main loop ----------------
            prev = None  # state of the previous pass, division pending
            first = True
            for ici in range(nic):
                for hp in range(2):
                    av_ps = av_pool.tile([128, 512], f32, tag="av")
                    st = (av_ps, ici, hp)
                    pending = []
                    for jt in range(jt_n):
                        if first:
                            # interleave the remaining projections into the
                            # first pass, just in time for their consumers
                            if jt % 4 == 0 and jt > 0:
                                kproj(0, jt // 4)
                            if jt % 8 == 4:
                                kproj(1, jt // 8 * 2)
                                kproj(1, jt // 8 * 2 + 1)
                            if jt == 2:
                                qproj(1, 0)
                            if jt in (8, 16, 24):
                                qproj(0, jt // 8)
                                qproj(1, jt // 8)
                        if prev is not None and jt == DEFER:
                            epilogue_div(prev)
                            for pjt, pex in pending:
                                av_mms(st, pjt, pex)
                            pending = []
                        if prev is not None and jt == DEFER + 2:
                            if prev[2] == 1:  # second pass done -> project
                                epilogue_out(prev[1])
                            prev = None
                        ex = sim_exp(st, jt)
                        if first:
                            vproj(jt)
                        if prev is not None and jt < DEFER:
                            pending.append((jt, ex))
                        else:
                            av_mms(st, jt, ex)
                    prev = st
                    first = False
            epilogue_div(prev)
            epilogue_out(prev[1])


def _host_prep(w_qkv, w_out, b_out):
    scale = DH**-0.5
    w_qkv = np.asarray(w_qkv, dtype=np.float32)
    wq = w_qkv[0:HID] * scale
    wk = w_qkv[HID : 2 * HID]
    wv = w_qkv[2 * HID :]
    # planes: plane p holds heads {2p, 2p+1}; within a plane the even slot
    # sits at psum partitions 0-31 (array rows 0-31) and the odd slot at
    # partitions 64-95 (rows 64-95); rows 32-63/96-127 are zero.
    w_qkvt = np.zeros((DIM, 5 * HID), np.float32)  # [256, 640]
    for p in range(2):
        for s_ in range(2):
            h = 2 * p + s_
            w_qkvt[:, 128 * p + 64 * s_ : 128 * p + 64 * s_ + 32] = wq[
                32 * h : 32 * h + 32
            ].T
            w_qkvt[:, 256 + 128 * p + 64 * s_ : 256 + 128 * p + 64 * s_ + 32] = wk[
                32 * h : 32 * h + 32
            ].T
    w_qkvt[:, 4 * HID :] = wv.T
    w_qkvt = np.ascontiguousarray(w_qkvt).astype(BF16)
    w_outT = np.asarray(w_out, dtype=np.float32).T  # [128, 256]
    wouta = np.zeros((128, 256), np.float32)
    woutb = np.zeros((128, 256), np.float32)
    wouta[0:32] = w_outT[0:32]  # head 0
    wouta[64:96] = w_outT[32:64]  # head 1
    woutb[0:32] = w_outT[64:96]  # head 2
    woutb[64:96] = w_outT[96:128]  # head 3
    b2 = np.ascontiguousarray(
        np.asarray(b_out, dtype=np.float32).reshape(2, 128).T
    )  # [128, 2]
    return w_qkvt, wouta.astype(BF16), woutb.astype(BF16), b2


def _build_program(repeat=1):
    import concourse.tile as tile
    from concourse import bacc, mybir

    f32 = mybir.dt.float32
    bf16 = mybir.dt.bfloat16
    nc = bacc.Bacc("TRN2", target_bir_lowering=False, debug=False)
    x_kv_h = nc.declare_dram_parameter("x_kv", [DIM, NJ], bf16, isOutput=False)
    x_q_h = nc.declare_dram_parameter("x_q", [DIM, NI], bf16, isOutput=False)
    w_qkvt_h = nc.declare_dram_parameter("w_qkvt", [DIM, 5 * HID], bf16, isOutput=False)
    wouta_h = nc.declare_dram_parameter("w_out_a", [128, 256], bf16, isOutput=False)
    woutb_h = nc.declare_dram_parameter("w_out_b", [128, 256], bf16, isOutput=False)
    b2_h = nc.declare_dram_parameter("b2", [128, 2], f32, isOutput=False)
    out_h = nc.declare_dram_parameter("out", [DIM, NI], f32, isOutput=True)

    in_aps = {
        "x_kv": x_kv_h[:],
        "x_q": x_q_h[:],
        "w_qkvt": w_qkvt_h[:],
        "w_out_a": wouta_h[:],
        "w_out_b": woutb_h[:],
        "b2": b2_h[:],
    }
    with tile.TileContext(nc) as tc:
        build_attn(tc, out_h[:], in_aps, NJ, NI, IC, repeat=repeat)
    nc.compile()
    return nc


def _make_in_maps(x, w_qkv, w_out, b_out):
    w_qkvt, wouta, woutb, b2 = _host_prep(w_qkv, w_out, b_out)
    xf = np.asarray(x, dtype=np.float32).reshape(B, DIM, NJ).astype(BF16)
    in_maps = []
    for c in range(N_CORES):
        b, half = c // 2, c % 2
        in_maps.append(
            {
                "x_kv": np.ascontiguousarray(xf[b]),
                "x_q": np.ascontiguousarray(xf[b][:, half * NI : (half + 1) * NI]),
                "w_qkvt": w_qkvt,
                "w_out_a": wouta,
                "w_out_b": woutb,
                "b2": b2,
            }
        )
    return in_maps


def _assemble(results):
    out_full = np.empty((B, DIM, NJ), np.float32)
    for c in range(N_CORES):
        b, half = c // 2, c % 2
        out_full[b][:, half * NI : (half + 1) * NI] = results[c]["out"]
    return out_full.reshape(B, DIM, 64, 64)


def _run_spmd(x, w_qkv, w_out, b_out, trace=False):
    from concourse.bass_utils import run_bass_kernel_spmd

    nc = _build_program()
    in_maps = _make_in_maps(x, w_qkv, w_out, b_out)
    res = run_bass_kernel_spmd(nc, in_maps, list(range(N_CORES)), trace=trace)
    return _assemble(res.results), res


def kernel(**inputs):
    out, _ = _run_spmd(
        inputs["x"], inputs["w_qkv"], inputs["w_out"], inputs["b_out"]
    )
    return out


# revision 8
# speedup vs baseline: 1.1672x; 1.1672x over previous
"""Trainium2 Bass kernel: multi-head self-attention over images (1x1-conv QKV).

Problem: x [4, 256, 64, 64], w_qkv [384, 256], w_out [256, 128], b_out [256].
  qkv = w_qkv @ x_flat ; per-head (h=4, d=32) softmax attention over n=4096 ;
  out = w_out @ heads + b_out.

Sharding across 8 cores: (batch, query-half) pairs -> each core handles one
batch's K/V over the full 4096 positions and attention + output projection
for 2048 of its query positions.  Outputs are disjoint slices; the host
only concatenates.

Per-core dataflow (v3 = head-pair passes, N=512 matmuls):
  - All matmul operands are bf16 (fp32 costs 4 cycles/row on the PE, bf16
    one); accumulation stays fp32 in PSUM.  Host pre-converts inputs.
  - Work is split into 4 i-chunks of 512 columns, each processed in two
    head-pair passes (plane 0 = heads 0,1; plane 1 = heads 2,3).  Per
    (pass, j-tile): 2 sim matmuls [K=32 x N=512] at array rows 0-31/64-95
    writing the two banks of a [128, 2, 512] PSUM tile, one exp on ScalarE
    (N=1024, PSUM -> SBUF bf16), 2 AV matmuls accumulating into a single
    av bank at partition rows 0-32 / 64-96 (ones column at d=32 gives the
    softmax denominator).  N=512 halves the PE instruction count vs an
    ic=256 layout - HW probes showed per-instruction overheads on PE and
    ACT are first-order costs.
  - Softmax division: row-sums broadcast across partitions with K=1
    matmuls against a constant mask, reciprocal + multiply on VectorE.
  - Per chunk the output projection accumulates both head-pair passes
    (host-prescrambled w_out halves) + bias.
  - Prologue K/Q/V projections and input DMAs are split and interleaved
    into the first pass so ScalarE starts ~3us in; each pass's division
    is deferred into the next pass's first j-tiles so ScalarE never idles.
"""

import sys

import numpy as np

for _p in ("/opt/trn_rl_repo",):
    if _p not in sys.path:
        sys.path.insert(0, _p)

import ml_dtypes

BF16 = ml_dtypes.bfloat16

HEADS = 4
DH = 32
DIM = 256
HID = HEADS * DH  # 128
B = 4
N_CORES = 8
NJ = 4096  # full context per batch (64*64)
NI = NJ // 2  # queries per core
IC = 512  # i-chunk (columns per inner tile)


def build_attn(tc, out_ap, in_aps, nj, ni, ic, repeat=1):
    """Emit the per-core attention program.

    out_ap: DRAM AP [256, ni] fp32
    in_aps: dict with DRAM APs:
        x_kv  [256, nj]   bf16 batch image, channels-major
        x_q   [256, ni]   bf16 this core's query columns
        w_qkvt [256, 640] bf16 (w_qkv with q-scale folded in).T, prescrambled
        w_out_a/w_out_b [128, 256] bf16 prescrambled output weights
        b2    [128, 2]    fp32 bias, b2[p, mc] = b_out[mc*128 + p]
    """
    import concourse.tile as tile  # noqa: F401
    from concourse import mybir

    nc = tc.nc
    f32 = mybir.dt.float32
    bf16 = mybir.dt.bfloat16
    Exp = mybir.ActivationFunctionType.Exp
    jt_n = nj // 128
    nic = ni // ic
    assert nj % 512 == 0 and ni % 512 == 0 and ni % ic == 0

    from contextlib import ExitStack

    with ExitStack() as ctx:
        const = ctx.enter_context(tc.tile_pool(name="const", bufs=1))
        sim_pool = ctx.enter_context(tc.tile_pool(name="simps", bufs=2, space="PSUM"))
        av_pool = ctx.enter_context(tc.tile_pool(name="avps", bufs=2, space="PSUM"))
        work_pool = ctx.enter_context(tc.tile_pool(name="workps", bufs=2, space="PSUM"))
        exp_pool = ctx.enter_context(tc.tile_pool(name="expsb", bufs=7))
        sb_pool = ctx.enter_context(tc.tile_pool(name="sb", bufs=2))

        # ---------------- persistent SBUF ----------------
        x_sb = const.tile([128, 2, nj], bf16, tag="x_sb")
        xq_sb = const.tile([128, 2, ni], bf16, tag="xq_sb")
        wqkvt_sb = const.tile([128, 2, 5 * HID], bf16, tag="wqkvt")
        wouta_sb = const.tile([128, 256], bf16, tag="wouta")
        woutb_sb = const.tile([128, 256], bf16, tag="woutb")
        b_sb = const.tile([128, 2], f32, tag="b_sb")
        mask_sb = const.tile([128, 128], bf16, tag="mask")
        k_sb = const.tile([128, 2, nj], bf16, tag="k_sb")
        q_sb = const.tile([128, 2, ni], bf16, tag="q_sb")
        vt_sb = const.tile([128, jt_n, HEADS, DH + 1], bf16, tag="vt_sb")
        # normalized head outputs; [hd, buf, pass, ic]; rows 33-63/97-127 stay 0
        oh_sb = const.tile([128, 2, 2, ic], bf16, tag="oh_sb")

        # ---------------- input DMAs (split + ordered so compute starts
        # early: qkv weights and the first x/xq chunks first) -------------
        xr = in_aps["x_kv"].rearrange("(c p) n -> p c n", p=128)
        xqr = in_aps["x_q"].rearrange("(c p) n -> p c n", p=128)

        def xdma(jc):
            s = slice(jc * 512, (jc + 1) * 512)
            nc.sync.dma_start(x_sb[:, :, s], xr[:, :, s])

        def xqdma(qc):
            s = slice(qc * 512, (qc + 1) * 512)
            nc.sync.dma_start(xq_sb[:, :, s], xqr[:, :, s])

        nc.sync.dma_start(
            wqkvt_sb[:], in_aps["w_qkvt"].rearrange("(c p) m -> p c m", p=128)
        )
        xdma(0)
        xqdma(0)
        nc.sync.dma_start(wouta_sb[:], in_aps["w_out_a"][:])
        nc.sync.dma_start(woutb_sb[:], in_aps["w_out_b"][:])
        nc.sync.dma_start(b_sb[:], in_aps["b2"][:])
        for jc in range(1, nj // 512):
            xdma(jc)
        for qc in range(1, ni // 512):
            xqdma(qc)

        out_r = out_ap.rearrange("(m p) n -> p m n", p=128)

        # ---------------- emission helpers ----------------
        def kproj(pl, jc):
            ps = work_pool.tile([128, 2, 256], f32, tag="work")
            psf = ps[:].rearrange("p a b -> p (a b)")
            for cs in range(2):
                nc.tensor.matmul(
                    psf,
                    lhsT=wqkvt_sb[:, cs, 2 * HID + 128 * pl : 2 * HID + 128 * (pl + 1)],
                    rhs=x_sb[:, cs, jc * 512 : (jc + 1) * 512],
                    start=(cs == 0),
                    stop=(cs == 1),
                )
            nc.vector.tensor_copy(k_sb[:, pl, jc * 512 : (jc + 1) * 512], psf)

        def qproj(pl, qc):
            ps = work_pool.tile([128, 2, 256], f32, tag="work")
            psf = ps[:].rearrange("p a b -> p (a b)")
            for cs in range(2):
                nc.tensor.matmul(
                    psf,
                    lhsT=wqkvt_sb[:, cs, 128 * pl : 128 * (pl + 1)],
                    rhs=xq_sb[:, cs, qc * 512 : (qc + 1) * 512],
                    start=(cs == 0),
                    stop=(cs == 1),
                )
            nc.vector.tensor_copy(q_sb[:, pl, qc * 512 : (qc + 1) * 512], psf)

        def vproj(jt):
            # V^T: vt_sb[j, h, d] (+ ones column at d=DH)
            ps = work_pool.tile([128, 2, 256], f32, tag="work")
            psf = ps[:].rearrange("p a b -> p (a b)")
            for cs in range(2):
                nc.tensor.matmul(
                    psf[:, 0:HID],
                    lhsT=x_sb[:, cs, jt * 128 : (jt + 1) * 128],
                    rhs=wqkvt_sb[:, cs, 4 * HID : 5 * HID],
                    start=(cs == 0),
                    stop=(cs == 1),
                )
            nc.vector.tensor_copy(
                vt_sb[:, jt, :, 0:DH],
                psf[:, 0:HID].rearrange("p (h d) -> p h d", h=HEADS),
            )

        def sim_exp(st, jt):
            """sim matmuls + exp for one j-tile of one pass; returns exp tile."""
            av_ps, ici, hp = st
            i_sl = slice(ici * ic, (ici + 1) * ic)
            sim_ps = sim_pool.tile([128, 2, ic], f32, tag="sim")
            # the two heads of the pass hit different PSUM banks at array
            # row positions 0 and 64, so they run concurrently.
            for s in range(2):
                nc.tensor.matmul(
                    sim_ps[:, s, :],
                    lhsT=k_sb[64 * s : 64 * s + 32, hp, jt * 128 : (jt + 1) * 128],
                    rhs=q_sb[64 * s : 64 * s + 32, hp, i_sl],
                    start=True,
                    stop=True,
                    tile_position=(64 * s, 0),
                )
            ex = exp_pool.tile([128, 2, ic], bf16, tag="exp")
            nc.scalar.activation(ex[:], sim_ps[:], Exp)
            return ex

        def av_mms(st, jt, ex):
            av_ps, ici, hp = st
            for s in range(2):
                pos = 64 * s
                # the two accumulation groups live at disjoint partition
                # ranges of one bank; the sim's group check is partition-
                # blind, so it must be skipped (HW-legal).
                nc.tensor.matmul(
                    av_ps[pos : pos + DH + 1, 0:ic],
                    lhsT=vt_sb[:, jt, 2 * hp + s, :],
                    rhs=ex[:, s, :],
                    start=(jt == 0),
                    stop=(jt == jt_n - 1),
                    skip_group_check=True,
                )

        def epilogue_div(st):
            """Softmax division: av -> normalized oh_sb (frees the av bank)."""
            av_ps, ici, hp = st
            buf = ici % 2
            sums = sb_pool.tile([128, ic], bf16, tag="sums")
            nc.vector.tensor_copy(sums[32:33], av_ps[32:33, 0:ic])
            nc.vector.tensor_copy(sums[96:97], av_ps[96:97, 0:ic])
            bc_e = work_pool.tile([128, 2, 256], f32, tag="work")
            bc_o = work_pool.tile([128, 2, 256], f32, tag="work")
            bce = bc_e[:].rearrange("p a b -> p (a b)")
            bco = bc_o[:].rearrange("p a b -> p (a b)")
            nc.tensor.matmul(
                bce,
                lhsT=mask_sb[32:33, :],
                rhs=sums[32:33, :],
                start=True,
                stop=True,
                tile_position=(32, 0),
            )
            nc.tensor.matmul(
                bco,
                lhsT=mask_sb[96:97, :],
                rhs=sums[96:97, :],
                start=True,
                stop=True,
                tile_position=(96, 0),
            )
            recip = sb_pool.tile([128, ic], f32, tag="recip")
            nc.vector.reciprocal(recip[0:33], bce[0:33])
            nc.vector.reciprocal(recip[64:97], bco[64:97])
            nc.vector.tensor_mul(oh_sb[0:33, buf, hp], av_ps[0:33, 0:ic], recip[0:33])
            nc.vector.tensor_mul(oh_sb[64:97, buf, hp], av_ps[64:97, 0:ic], recip[64:97])

        def epilogue_out(ici):
            """Output projection + bias + store for chunk ici (both passes)."""
            buf = ici % 2
            for mc in range(2):
                pout = work_pool.tile([128, 2, 256], f32, tag="work")
                pf = pout[:].rearrange("p a b -> p (a b)")
                nc.tensor.matmul(
                    pf,
                    lhsT=wouta_sb[:, mc * 128 : (mc + 1) * 128],
                    rhs=oh_sb[:, buf, 0, :],
                    start=True,
                    stop=False,
                )
                nc.tensor.matmul(
                    pf,
                    lhsT=woutb_sb[:, mc * 128 : (mc + 1) * 128],
                    rhs=oh_sb[:, buf, 1, :],
                    start=False,
                    stop=True,
                )
                final = sb_pool.tile([128, ic], f32, tag="final")
                nc.vector.tensor_scalar_add(final[:], pf, b_sb[:, mc : mc + 1])
                nc.sync.dma_start(
                    out_r[:, mc, ici * ic : (ici + 1) * ic], final[:]
                )

        DEFER = 6  # j-tiles of AV deferred past the previous pass's division

        # ---------------- constants (persistent across reps) ----------------
        nc.vector.memset(mask_sb[:], 0.0)
        nc.vector.memset(mask_sb[32:33, 0:33], 1.0)
        nc.vector.memset(mask_sb[96:97, 64:97], 1.0)
        nc.vector.memset(vt_sb[:, :, :, DH : DH + 1], 1.0)
        # rows 33-63 / 97-127 stay zero; live rows are rewritten every chunk
        nc.vector.memset(oh_sb[:], 0.0)

        for _rep in range(repeat):

            # minimal prologue: K and Q for the first j/i chunks only; the
            # rest is interleaved into the first pass so ScalarE starts early.
            kproj(0, 0)
            qproj(0, 0)

            # ---------------- main loop ----------------
            prev = None  # state of the previous pass, division pending
            first = True
            for ici in range(nic):
                for hp in range(2):
                    av_ps = av_pool.tile([128, 512], f32, tag="av")
                    st = (av_ps, ici, hp)
                    pending = []
                    for jt in range(jt_n):
                        if first:
                            # interleave the remaining projections into the
                            # first pass, just in time for their consumers
                            if jt % 4 == 0 and jt > 0:
                                kproj(0, jt // 4)
                            if jt % 8 == 4:
                                kproj(1, jt // 8 * 2)
                                kproj(1, jt // 8 * 2 + 1)
                            if jt == 2:
                                qproj(1, 0)
                            if jt in (8, 16, 24):
                                qproj(0, jt // 8)
                                qproj(1, jt // 8)
                        if prev is not None and jt == DEFER:
                            epilogue_div(prev)
                            for pjt, pex in pending:
                                av_mms(st, pjt, pex)
                            pending = []
                        if prev is not None and jt == DEFER + 2:
                            if prev[2] == 1:  # second pass done -> project
                                epilogue_out(prev[1])
                            prev = None
                        ex = sim_exp(st, jt)
                        if first:
                            vproj(jt)
                        if prev is not None and jt < DEFER:
                            pending.append((jt, ex))
                        else:
                            av_mms(st, jt, ex)
                    prev = st
                    first = False
            epilogue_div(prev)
            epilogue_out(prev[1])


def _host_prep(w_qkv, w_out, b_out):
    scale = DH**-0.5
    w_qkv = np.asarray(w_qkv, dtype=np.float32)
    wq = w_qkv[0:HID] * scale
    wk = w_qkv[HID : 2 * HID]
    wv = w_qkv[2 * HID :]
    # planes: plane p holds heads {2p, 2p+1}; within a plane the even slot
    # sits at psum partitions 0-31 (array rows 0-31) and the odd slot at
    # partitions 64-95 (rows 64-95); rows 32-63/96-127 are zero.
    w_qkvt = np.zeros((DIM, 5 * HID), np.float32)  # [256, 640]
    for p in range(2):
        for s_ in range(2):
            h = 2 * p + s_
            w_qkvt[:, 128 * p + 64 * s_ : 128 * p + 64 * s_ + 32] = wq[
                32 * h : 32 * h + 32
            ].T
            w_qkvt[:, 256 + 128 * p + 64 * s_ : 256 + 128 * p + 64 * s_ + 32] = wk[
                32 * h : 32 * h + 32
            ].T
    w_qkvt[:, 4 * HID :] = wv.T
    w_qkvt = np.ascontiguousarray(w_qkvt).astype(BF16)
    w_outT = np.asarray(w_out, dtype=np.float32).T  # [128, 256]
    wouta = np.zeros((128, 256), np.float32)
    woutb = np.zeros((128, 256), np.float32)
    wouta[0:32] = w_outT[0:32]  # head 0
    wouta[64:96] = w_outT[32:64]  # head 1
    woutb[0:32] = w_outT[64:96]  # head 2
    woutb[64:96] = w_outT[96:128]  # head 3
    b2 = np.ascontiguousarray(
        np.asarray(b_out, dtype=np.float32).reshape(2, 128).T
    )  # [128, 2]
    return w_qkvt, wouta.astype(BF16), woutb.astype(BF16), b2


def _build_program(repeat=1):
    import concourse.tile as tile
    from concourse import bacc, mybir

    f32 = mybir.dt.float32
    bf16 = mybir.dt.bfloat16
    nc = bacc.Bacc("TRN2", target_bir_lowering=False, debug=False)
    x_kv_h = nc.declare_dram_parameter("x_kv", [DIM, NJ], bf16, isOutput=False)
    x_q_h = nc.declare_dram_parameter("x_q", [DIM, NI], bf16, isOutput=False)
    w_qkvt_h = nc.declare_dram_parameter("w_qkvt", [DIM, 5 * HID], bf16, isOutput=False)
    wouta_h = nc.declare_dram_parameter("w_out_a", [128, 256], bf16, isOutput=False)
    woutb_h = nc.declare_dram_parameter("w_out_b", [128, 256], bf16, isOutput=False)
    b2_h = nc.declare_dram_parameter("b2", [128, 2], f32, isOutput=False)
    out_h = nc.declare_dram_parameter("out", [DIM, NI], f32, isOutput=True)

    in_aps = {
        "x_kv": x_kv_h[:],
        "x_q": x_q_h[:],
        "w_qkvt": w_qkvt_h[:],
        "w_out_a": wouta_h[:],
        "w_out_b": woutb_h[:],
        "b2": b2_h[:],
    }
    with tile.TileContext(nc) as tc:
        build_attn(tc, out_h[:], in_aps, NJ, NI, IC, repeat=repeat)
    nc.compile()
    return nc


def _make_in_maps(x, w_qkv, w_out, b_out):
    w_qkvt, wouta, woutb, b2 = _host_prep(w_qkv, w_out, b_out)
    xf = np.asarray(x, dtype=np.float32).reshape(B, DIM, NJ).astype(BF16)
    in_maps = []
    for c in range(N_CORES):
        b, half = c // 2, c % 2
        in_maps.append(
            {
                "x_kv": np.ascontiguousarray(xf[b]),
                "x_q": np.ascontiguousarray(xf[b][:, half * NI : (half + 1) * NI]),
                "w_qkvt": w_qkvt,
                "w_out_a": wouta,
                "w_out_b": woutb,
                "b2": b2,
            }
        )
    return in_maps


def _assemble(results):
    out_full = np.empty((B, DIM, NJ), np.float32)
    for c in range(N_CORES):
        b, half = c // 2, c % 2
        out_full[b][:, half * NI : (half + 1) * NI] = results[c]["out"]
    return out_full.reshape(B, DIM, 64, 64)


def _run_spmd(x, w_qkv, w_out, b_out, trace=False):
    from concourse.bass_utils import run_bass_kernel_spmd

    nc = _build_program()
    in_maps = _make_in_maps(x, w_qkv, w_out, b_out)
    res = run_bass_kernel_spmd(nc, in_maps, list(range(N_CORES)), trace=trace)
    return _assemble(res.results), res


def kernel(**inputs):
    out, _ = _run_spmd(
        inputs["x"], inputs["w_qkv"], inputs["w_out"], inputs["b_out"]
    )
    return out


# revision 9
# speedup vs baseline: 1.2860x; 1.1018x over previous
"""Trainium2 Bass kernel: multi-head self-attention over images (1x1-conv QKV).

Problem: x [4, 256, 64, 64], w_qkv [384, 256], w_out [256, 128], b_out [256].
  qkv = w_qkv @ x_flat ; per-head (h=4, d=32) softmax attention over n=4096 ;
  out = w_out @ heads + b_out.

Sharding across 8 cores: (batch, query-half) pairs -> each core handles one
batch's K/V over the full 4096 positions and attention + output projection
for 2048 of its query positions.  Outputs are disjoint slices; the host
only concatenates.

Per-core dataflow (v3 = head-pair passes, N=512 matmuls):
  - All matmul operands are bf16 (fp32 costs 4 cycles/row on the PE, bf16
    one); accumulation stays fp32 in PSUM.  Host pre-converts inputs.
  - Work is split into 4 i-chunks of 512 columns, each processed in two
    head-pair passes (plane 0 = heads 0,1; plane 1 = heads 2,3).  Per
    (pass, j-tile): 2 sim matmuls [K=32 x N=512] at array rows 0-31/64-95
    writing the two banks of a [128, 2, 512] PSUM tile, one exp on ScalarE
    (N=1024, PSUM -> SBUF bf16), 2 AV matmuls accumulating into a single
    av bank at partition rows 0-32 / 64-96 (ones column at d=32 gives the
    softmax denominator).  N=512 halves the PE instruction count vs an
    ic=256 layout - HW probes showed per-instruction overheads on PE and
    ACT are first-order costs.
  - Softmax division: row-sums broadcast across partitions with K=1
    matmuls against a constant mask, reciprocal + multiply on VectorE.
  - Per chunk the output projection accumulates both head-pair passes
    (host-prescrambled w_out halves) + bias.
  - Prologue K/Q/V projections and input DMAs are split and interleaved
    into the first pass so ScalarE starts ~3us in; each pass's division
    is deferred into the next pass's first j-tiles so ScalarE never idles.
"""

import sys

import numpy as np

for _p in ("/opt/trn_rl_repo",):
    if _p not in sys.path:
        sys.path.insert(0, _p)

import ml_dtypes

BF16 = ml_dtypes.bfloat16

HEADS = 4
DH = 32
DIM = 256
HID = HEADS * DH  # 128
B = 4
N_CORES = 8
NJ = 4096  # full context per batch (64*64)
NI = NJ // 2  # queries per core
IC = 512  # i-chunk (columns per inner tile)


def build_attn(tc, out_ap, in_aps, nj, ni, ic, repeat=1):
    """Emit the per-core attention program.

    out_ap: DRAM AP [256, ni] fp32
    in_aps: dict with DRAM APs:
        x_kv  [256, nj]   bf16 batch image, channels-major
        x_q   [256, ni]   bf16 this core's query columns
        w_qkvt [256, 640] bf16 (w_qkv with q-scale folded in).T, prescrambled
        w_out_a/w_out_b [128, 256] bf16 prescrambled output weights
        b2    [128, 2]    fp32 bias, b2[p, mc] = b_out[mc*128 + p]
    """
    import concourse.tile as tile  # noqa: F401
    from concourse import mybir

    nc = tc.nc
    f32 = mybir.dt.float32
    bf16 = mybir.dt.bfloat16
    Exp = mybir.ActivationFunctionType.Exp
    jt_n = nj // 128
    nic = ni // ic
    assert nj % 512 == 0 and ni % 512 == 0 and ni % ic == 0

    from contextlib import ExitStack

    with ExitStack() as ctx:
        const = ctx.enter_context(tc.tile_pool(name="const", bufs=1))
        sim_pool = ctx.enter_context(tc.tile_pool(name="simps", bufs=2, space="PSUM"))
        av_pool = ctx.enter_context(tc.tile_pool(name="avps", bufs=2, space="PSUM"))
        work_pool = ctx.enter_context(tc.tile_pool(name="workps", bufs=2, space="PSUM"))
        exp_pool = ctx.enter_context(tc.tile_pool(name="expsb", bufs=7))
        sb_pool = ctx.enter_context(tc.tile_pool(name="sb", bufs=2))

        # ---------------- persistent SBUF ----------------
        x_sb = const.tile([128, 2, nj], bf16, tag="x_sb")
        xq_sb = const.tile([128, 2, ni], bf16, tag="xq_sb")
        wqkvt_sb = const.tile([128, 2, 5 * HID], bf16, tag="wqkvt")
        wouta_sb = const.tile([128, 256], bf16, tag="wouta")
        woutb_sb = const.tile([128, 256], bf16, tag="woutb")
        b_sb = const.tile([128, 2], f32, tag="b_sb")
        mask_sb = const.tile([128, 128], bf16, tag="mask")
        k_sb = const.tile([128, 2, nj], bf16, tag="k_sb")
        q_sb = const.tile([128, 2, ni], bf16, tag="q_sb")
        vt_sb = const.tile([128, jt_n, HEADS, DH + 1], bf16, tag="vt_sb")
        # normalized head outputs; [hd, buf, pass, ic]; rows 33-63/97-127 stay 0
        oh_sb = const.tile([128, 2, 2, ic], bf16, tag="oh_sb")

        # ---------------- input DMAs (split + ordered so compute starts
        # early: qkv weights and the first x/xq chunks first) -------------
        xr = in_aps["x_kv"].rearrange("(c p) n -> p c n", p=128)
        xqr = in_aps["x_q"].rearrange("(c p) n -> p c n", p=128)

        def xdma(jc):
            s = slice(jc * 512, (jc + 1) * 512)
            nc.sync.dma_start(x_sb[:, :, s], xr[:, :, s])

        def xqdma(qc):
            s = slice(qc * 512, (qc + 1) * 512)
            nc.sync.dma_start(xq_sb[:, :, s], xqr[:, :, s])

        nc.sync.dma_start(
            wqkvt_sb[:], in_aps["w_qkvt"].rearrange("(c p) m -> p c m", p=128)
        )
        xdma(0)
        xqdma(0)
        nc.sync.dma_start(wouta_sb[:], in_aps["w_out_a"][:])
        nc.sync.dma_start(woutb_sb[:], in_aps["w_out_b"][:])
        nc.sync.dma_start(b_sb[:], in_aps["b2"][:])
        for jc in range(1, nj // 512):
            xdma(jc)
        for qc in range(1, ni // 512):
            xqdma(qc)

        out_r = out_ap.rearrange("(m p) n -> p m n", p=128)

        # ---------------- emission helpers ----------------
        def kproj(pl, jc):
            ps = work_pool.tile([128, 2, 256], f32, tag="work")
            psf = ps[:].rearrange("p a b -> p (a b)")
            for cs in range(2):
                nc.tensor.matmul(
                    psf,
                    lhsT=wqkvt_sb[:, cs, 2 * HID + 128 * pl : 2 * HID + 128 * (pl + 1)],
                    rhs=x_sb[:, cs, jc * 512 : (jc + 1) * 512],
                    start=(cs == 0),
                    stop=(cs == 1),
                )
            nc.vector.tensor_copy(k_sb[:, pl, jc * 512 : (jc + 1) * 512], psf)

        def qproj(pl, qc):
            ps = work_pool.tile([128, 2, 256], f32, tag="work")
            psf = ps[:].rearrange("p a b -> p (a b)")
            for cs in range(2):
                nc.tensor.matmul(
                    psf,
                    lhsT=wqkvt_sb[:, cs, 128 * pl : 128 * (pl + 1)],
                    rhs=xq_sb[:, cs, qc * 512 : (qc + 1) * 512],
                    start=(cs == 0),
                    stop=(cs == 1),
                )
            nc.vector.tensor_copy(q_sb[:, pl, qc * 512 : (qc + 1) * 512], psf)

        def vproj(jt):
            # V^T: vt_sb[j, h, d] (+ ones column at d=DH)
            ps = work_pool.tile([128, 2, 256], f32, tag="work")
            psf = ps[:].rearrange("p a b -> p (a b)")
            for cs in range(2):
                nc.tensor.matmul(
                    psf[:, 0:HID],
                    lhsT=x_sb[:, cs, jt * 128 : (jt + 1) * 128],
                    rhs=wqkvt_sb[:, cs, 4 * HID : 5 * HID],
                    start=(cs == 0),
                    stop=(cs == 1),
                )
            nc.vector.tensor_copy(
                vt_sb[:, jt, :, 0:DH],
                psf[:, 0:HID].rearrange("p (h d) -> p h d", h=HEADS),
            )

        def sim_exp(st, jt):
            """sim matmuls + exp for one j-tile of one pass; returns exp tile."""
            av_ps, ici, hp = st
            i_sl = slice(ici * ic, (ici + 1) * ic)
            sim_ps = sim_pool.tile([128, 2, ic], f32, tag="sim")
            # the two heads of the pass hit different PSUM banks at array
            # row positions 0 and 64, so they run concurrently.
            for s in range(2):
                nc.tensor.matmul(
                    sim_ps[:, s, :],
                    lhsT=k_sb[64 * s : 64 * s + 32, hp, jt * 128 : (jt + 1) * 128],
                    rhs=q_sb[64 * s : 64 * s + 32, hp, i_sl],
                    start=True,
                    stop=True,
                    tile_position=(64 * s, 0),
                )
            ex = exp_pool.tile([128, 2, ic], bf16, tag="exp")
            nc.scalar.activation(ex[:], sim_ps[:], Exp)
            return ex

        def av_mms(st, jt, ex):
            av_ps, ici, hp = st
            for s in range(2):
                pos = 64 * s
                # the two accumulation groups live at disjoint partition
                # ranges of one bank; the sim's group check is partition-
                # blind, so it must be skipped (HW-legal).
                nc.tensor.matmul(
                    av_ps[pos : pos + DH + 1, 0:ic],
                    lhsT=vt_sb[:, jt, 2 * hp + s, :],
                    rhs=ex[:, s, :],
                    start=(jt == 0),
                    stop=(jt == jt_n - 1),
                    skip_group_check=True,
                )

        def epilogue_div(st):
            """Softmax division: av -> normalized oh_sb (frees the av bank)."""
            av_ps, ici, hp = st
            buf = ici % 2
            sums = sb_pool.tile([128, ic], bf16, tag="sums")
            nc.vector.tensor_copy(sums[32:33], av_ps[32:33, 0:ic])
            nc.vector.tensor_copy(sums[96:97], av_ps[96:97, 0:ic])
            bc_e = work_pool.tile([128, 2, 256], f32, tag="work")
            bc_o = work_pool.tile([128, 2, 256], f32, tag="work")
            bce = bc_e[:].rearrange("p a b -> p (a b)")
            bco = bc_o[:].rearrange("p a b -> p (a b)")
            nc.tensor.matmul(
                bce,
                lhsT=mask_sb[32:33, :],
                rhs=sums[32:33, :],
                start=True,
                stop=True,
                tile_position=(32, 0),
            )
            nc.tensor.matmul(
                bco,
                lhsT=mask_sb[96:97, :],
                rhs=sums[96:97, :],
                start=True,
                stop=True,
                tile_position=(96, 0),
            )
            recip = sb_pool.tile([128, ic], f32, tag="recip")
            nc.vector.reciprocal(recip[0:33], bce[0:33])
            nc.vector.reciprocal(recip[64:97], bco[64:97])
            nc.vector.tensor_mul(oh_sb[0:33, buf, hp], av_ps[0:33, 0:ic], recip[0:33])
            nc.vector.tensor_mul(oh_sb[64:97, buf, hp], av_ps[64:97, 0:ic], recip[64:97])

        def epilogue_out(ici):
            """Output projection + bias + store for chunk ici (both passes)."""
            buf = ici % 2
            for mc in range(2):
                pout = work_pool.tile([128, 2, 256], f32, tag="work")
                pf = pout[:].rearrange("p a b -> p (a b)")
                nc.tensor.matmul(
                    pf,
                    lhsT=wouta_sb[:, mc * 128 : (mc + 1) * 128],
                    rhs=oh_sb[:, buf, 0, :],
                    start=True,
                    stop=False,
                )
                nc.tensor.matmul(
                    pf,
                    lhsT=woutb_sb[:, mc * 128 : (mc + 1) * 128],
                    rhs=oh_sb[:, buf, 1, :],
                    start=False,
                    stop=True,
                )
                final = sb_pool.tile([128, ic], f32, tag="final")
                nc.vector.tensor_scalar_add(final[:], pf, b_sb[:, mc : mc + 1])
                nc.sync.dma_start(
                    out_r[:, mc, ici * ic : (ici + 1) * ic], final[:]
                )

        DEFER = 6  # j-tiles of AV deferred past the previous pass's division

        # ---------------- constants (persistent across reps) ----------------
        nc.vector.memset(mask_sb[:], 0.0)
        nc.vector.memset(mask_sb[32:33, 0:33], 1.0)
        nc.vector.memset(mask_sb[96:97, 64:97], 1.0)
        nc.vector.memset(vt_sb[:, :, :, DH : DH + 1], 1.0)
        # rows 33-63 / 97-127 stay zero; live rows are rewritten every chunk
        nc.vector.memset(oh_sb[:], 0.0)

        for _rep in range(repeat):

            # minimal prologue: K and Q for the first j/i chunks only; the
            # rest is interleaved into the first pass so ScalarE starts early.
            kproj(0, 0)
            qproj(0, 0)

            # ---------------- main loop ----------------
            prev = None  # state of the previous pass, division pending
            first = True
            for ici in range(nic):
                for hp in range(2):
                    av_ps = av_pool.tile([128, 512], f32, tag="av")
                    st = (av_ps, ici, hp)
                    pending = []
                    for jt in range(jt_n):
                        if first:
                            # interleave the remaining projections into the
                            # first pass, just in time for their consumers
                            if jt % 4 == 0 and jt > 0:
                                kproj(0, jt // 4)
                            if jt % 8 == 4:
                                kproj(1, jt // 8 * 2)
                            if jt % 8 == 6:
                                kproj(1, jt // 8 * 2 + 1)
                            if jt == 2:
                                qproj(1, 0)
                            if jt in (8, 16, 24):
                                qproj(0, jt // 8)
                            if jt in (10, 18, 26):
                                qproj(1, jt // 8)
                        if prev is not None and jt == DEFER:
                            epilogue_div(prev)
                            for pjt, pex in pending:
                                av_mms(st, pjt, pex)
                            pending = []
                        if prev is not None and jt == DEFER + 2:
                            if prev[2] == 1:  # second pass done -> project
                                epilogue_out(prev[1])
                            prev = None
                        ex = sim_exp(st, jt)
                        if first:
                            vproj(jt)
                        if prev is not None and jt < DEFER:
                            pending.append((jt, ex))
                        else:
                            av_mms(st, jt, ex)
                    prev = st
                    first = False
            epilogue_div(prev)
            epilogue_out(prev[1])


def _host_prep(w_qkv, w_out, b_out):
    scale = DH**-0.5
    w_qkv = np.asarray(w_qkv, dtype=np.float32)
    wq = w_qkv[0:HID] * scale
    wk = w_qkv[HID : 2 * HID]
    wv = w_qkv[2 * HID :]
    # planes: plane p holds heads {2p, 2p+1}; within a plane the even slot
    # sits at psum partitions 0-31 (array rows 0-31) and the odd slot at
    # partitions 64-95 (rows 64-95); rows 32-63/96-127 are zero.
    w_qkvt = np.zeros((DIM, 5 * HID), np.float32)  # [256, 640]
    for p in range(2):
        for s_ in range(2):
            h = 2 * p + s_
            w_qkvt[:, 128 * p + 64 * s_ : 128 * p + 64 * s_ + 32] = wq[
                32 * h : 32 * h + 32
            ].T
            w_qkvt[:, 256 + 128 * p + 64 * s_ : 256 + 128 * p + 64 * s_ + 32] = wk[
                32 * h : 32 * h + 32
            ].T
    w_qkvt[:, 4 * HID :] = wv.T
    w_qkvt = np.ascontiguousarray(w_qkvt).astype(BF16)
    w_outT = np.asarray(w_out, dtype=np.float32).T  # [128, 256]
    wouta = np.zeros((128, 256), np.float32)
    woutb = np.zeros((128, 256), np.float32)
    wouta[0:32] = w_outT[0:32]  # head 0
    wouta[64:96] = w_outT[32:64]  # head 1
    woutb[0:32] = w_outT[64:96]  # head 2
    woutb[64:96] = w_outT[96:128]  # head 3
    b2 = np.ascontiguousarray(
        np.asarray(b_out, dtype=np.float32).reshape(2, 128).T
    )  # [128, 2]
    return w_qkvt, wouta.astype(BF16), woutb.astype(BF16), b2


def _build_program(repeat=1):
    import concourse.tile as tile
    from concourse import bacc, mybir

    f32 = mybir.dt.float32
    bf16 = mybir.dt.bfloat16
    nc = bacc.Bacc("TRN2", target_bir_lowering=False, debug=False)
    x_kv_h = nc.declare_dram_parameter("x_kv", [DIM, NJ], bf16, isOutput=False)
    x_q_h = nc.declare_dram_parameter("x_q", [DIM, NI], bf16, isOutput=False)
    w_qkvt_h = nc.declare_dram_parameter("w_qkvt", [DIM, 5 * HID], bf16, isOutput=False)
    wouta_h = nc.declare_dram_parameter("w_out_a", [128, 256], bf16, isOutput=False)
    woutb_h = nc.declare_dram_parameter("w_out_b", [128, 256], bf16, isOutput=False)
    b2_h = nc.declare_dram_parameter("b2", [128, 2], f32, isOutput=False)
    out_h = nc.declare_dram_parameter("out", [DIM, NI], f32, isOutput=True)

    in_aps = {
        "x_kv": x_kv_h[:],
        "x_q": x_q_h[:],
        "w_qkvt": w_qkvt_h[:],
        "w_out_a": wouta_h[:],
        "w_out_b": woutb_h[:],
        "b2": b2_h[:],
    }
    with tile.TileContext(nc) as tc:
        build_attn(tc, out_h[:], in_aps, NJ, NI, IC, repeat=repeat)
    nc.compile()
    return nc


def _make_in_maps(x, w_qkv, w_out, b_out):
    w_qkvt, wouta, woutb, b2 = _host_prep(w_qkv, w_out, b_out)
    xf = np.asarray(x, dtype=np.float32).reshape(B, DIM, NJ).astype(BF16)
    in_maps = []
    for c in range(N_CORES):
        b, half = c // 2, c % 2
        in_maps.append(
            {
                "x_kv": np.ascontiguousarray(xf[b]),
                "x_q": np.ascontiguousarray(xf[b][:, half * NI : (half + 1) * NI]),
                "w_qkvt": w_qkvt,
                "w_out_a": wouta,
                "w_out_b": woutb,
                "b2": b2,
            }
        )
    return in_maps


def _assemble(results):
    out_full = np.empty((B, DIM, NJ), np.float32)
    for c in range(N_CORES):
        b, half = c // 2, c % 2
        out_full[b][:, half * NI : (half + 1) * NI] = results[c]["out"]
    return out_full.reshape(B, DIM, 64, 64)


def _run_spmd(x, w_qkv, w_out, b_out, trace=False):
    from concourse.bass_utils import run_bass_kernel_spmd

    nc = _build_program()
    in_maps = _make_in_maps(x, w_qkv, w_out, b_out)
    res = run_bass_kernel_spmd(nc, in_maps, list(range(N_CORES)), trace=trace)
    return _assemble(res.results), res


def kernel(**inputs):
    out, _ = _run_spmd(
        inputs["x"], inputs["w_qkv"], inputs["w_out"], inputs["b_out"]
    )
    return out
